# revision 2
# baseline (speedup 1.0000x reference)
"""GCN layer (gather-scale-scatter + dense transform) on 8 trn2 NeuronCores.

out[r] = (sum_{e:row[e]=r} norm_e * x[col_e]  (incl self loop norm=dis^2)) @ W + bias
with norm_e = dis[row]*dis[col], dis = rsqrt(1 + outdeg).

Sharding: destination nodes partitioned across 8 cores (12500 each); each
core is fully independent (x is replicated input; no collectives).

Device algorithm per core:
  - edges grouped host-side by (dest block of 128, src block of 32768),
    padded to chunks of 128 edges, chunk counts equalized across cores so a
    single NEFF serves all 8 cores.
  - dma_gather pulls x[col] rows (512B) from HBM into SBUF slabs.
  - per chunk, DVE builds S[e,d] = (iota[d]==rowloc[e]) * norm[e] in one
    tensor_scalar op; PE accumulates aggT[f,d] += xg.T @ S in PSUM.
  - per dest block: aggT -> SBUF, out = aggT.T @ W + bias -> HBM.
"""

import numpy as np

import concourse.bass as bass
import concourse.mybir as mybir
import concourse.tile as tile
from concourse import bacc
from concourse.bass_utils import run_bass_kernel_spmd

F = 128          # feature dim (in == out)
CH = 128         # edges per chunk
N_CORES = 8


def _prep(x, edge_index, n_nodes, src_blk):
    """Host-side integer/index preprocessing. Returns per-core device arrays
    and the static chunk-structure shared by all cores."""
    npc = n_nodes // N_CORES            # nodes per core
    nblk = (npc + 127) // 128           # dest blocks per core
    nk = (n_nodes + src_blk - 1) // src_blk  # src blocks

    r = np.asarray(edge_index[0], dtype=np.int64)
    c = np.asarray(edge_index[1], dtype=np.int64)
    deg = (np.bincount(r, minlength=n_nodes) + 1).astype(np.float64)
    dis = (1.0 / np.sqrt(deg)).astype(np.float32)

    loops = np.arange(n_nodes, dtype=np.int64)
    rr = np.concatenate([r, loops])
    cc = np.concatenate([c, loops])
    norm = dis[rr] * dis[cc]

    core = rr // npc
    rloc = rr - core * npc
    b_arr = rloc >> 7                   # dest block
    d_arr = (rloc & 127).astype(np.float32)   # local dest within block
    k_arr = cc // src_blk

    # group counts per (core, b, k) -> equalized chunk counts Cbk [nblk, nk]
    gid = (core * nblk + b_arr) * nk + k_arr
    counts = np.bincount(gid, minlength=N_CORES * nblk * nk).reshape(
        N_CORES, nblk, nk
    )
    Cbk = np.ceil(counts / CH).astype(np.int64).max(axis=0)  # [nblk, nk]
    Ck_tot = Cbk.sum(axis=0)            # [nk] chunks per src block
    C_tot = int(Cbk.sum())

    # chunk base offsets
    chunk_base = np.zeros((nblk, nk), np.int64)   # in global (b-major) order
    chunk_base.reshape(-1)[1:] = np.cumsum(Cbk.reshape(-1))[:-1]
    kpos_base = np.zeros((nblk, nk), np.int64)    # per-k (b-major within k)
    kpos_base[1:, :] = np.cumsum(Cbk, axis=0)[:-1, :]

    per_core = []
    for ci in range(N_CORES):
        sel = core == ci
        bs, ks, ds, cs, ns = b_arr[sel], k_arr[sel], d_arr[sel], cc[sel], norm[sel]
        order = np.lexsort((cs, ks, bs))
        bs, ks, ds, cs, ns = bs[order], ks[order], ds[order], cs[order], ns[order]
        # position within (b,k) group
        g = bs * nk + ks
        cnt = np.bincount(g, minlength=nblk * nk)
        gstart = np.zeros(nblk * nk, np.int64)
        gstart[1:] = np.cumsum(cnt)[:-1]
        j = np.arange(len(g)) - gstart[g]

        rowloc = np.full(C_tot * CH, -1.0, np.float32)
        normv = np.zeros(C_tot * CH, np.float32)
        slot = chunk_base[bs, ks] * CH + j
        rowloc[slot] = ds
        normv[slot] = ns

        idx_ks = []
        for k in range(nk):
            arr = np.zeros(int(Ck_tot[k]) * CH, np.int16)
            m = ks == k
            kslot = kpos_base[bs[m], k] * CH + j[m]
            arr[kslot] = (cs[m] - k * src_blk).astype(np.int16)
            # wrap: idx i -> [i % 16, i // 16], replicated to 128 partitions
            wrapped = arr.reshape(-1, 16).T          # [16, Ck*8]
            idx_ks.append(np.tile(wrapped, (8, 1)).copy())  # [128, Ck*8]

        per_core.append(
            {
                "rowloc": rowloc.reshape(C_tot, CH).T.copy(),  # [128, C_tot]
                "normv": normv.reshape(C_tot, CH).T.copy(),
                **{f"idx{k}": idx_ks[k] for k in range(nk)},
            }
        )

    return per_core, Cbk, Ck_tot, C_tot, npc, nblk, nk


def _build(n_nodes, src_blk, Cbk, Ck_tot, C_tot, npc, nblk, nk, G):
    """Build the Bass program (shared by all cores)."""
    nc = bacc.Bacc(None, target_bir_lowering=False)
    dt = mybir.dt

    x_d = nc.dram_tensor("x", [n_nodes, F], dt.float32, kind="ExternalInput")
    w_d = nc.dram_tensor("w", [F, F], dt.float32, kind="ExternalInput")
    iota_d = nc.dram_tensor("iota", [128, 128], dt.float32, kind="ExternalInput")
    bias_d = nc.dram_tensor("biasrep", [128, F], dt.float32, kind="ExternalInput")
    rowloc_d = nc.dram_tensor("rowloc", [128, C_tot], dt.float32, kind="ExternalInput")
    normv_d = nc.dram_tensor("normv", [128, C_tot], dt.float32, kind="ExternalInput")
    idx_d = [
        nc.dram_tensor(f"idx{k}", [128, int(Ck_tot[k]) * 8], dt.int16,
                       kind="ExternalInput")
        for k in range(nk)
    ]
    y_d = nc.dram_tensor("y", [npc, F], dt.float32, kind="ExternalOutput")

    with tile.TileContext(nc) as tc:
        with (
            tc.tile_pool(name="const", bufs=1) as constp,
            tc.tile_pool(name="slab", bufs=10) as slabp,
            tc.tile_pool(name="idxp", bufs=10) as idxp,
            tc.tile_pool(name="sp", bufs=8) as sp,
            tc.tile_pool(name="pre", bufs=3) as prep,
            tc.tile_pool(name="ob", bufs=3) as obp,
            tc.tile_pool(name="pagg", bufs=4, space="PSUM") as paggp,
            tc.tile_pool(name="pout", bufs=2, space="PSUM") as poutp,
        ):
            w_sb = constp.tile([F, F], dt.float32, tag="w")
            nc.sync.dma_start(w_sb[:], w_d[:])
            iota_sb = constp.tile([128, 128], dt.float32, tag="iota")
            nc.sync.dma_start(iota_sb[:], iota_d[:])
            bias_sb = constp.tile([128, F], dt.float32, tag="bias")
            nc.sync.dma_start(bias_sb[:], bias_d[:])
            rowloc_sb = constp.tile([128, C_tot], dt.float32, tag="rowloc")
            nc.sync.dma_start(rowloc_sb[:], rowloc_d[:])
            normv_sb = constp.tile([128, C_tot], dt.float32, tag="normv")
            nc.sync.dma_start(normv_sb[:], normv_d[:])

            x_src = [x_d[k * src_blk: min((k + 1) * src_blk, n_nodes), :]
                     for k in range(nk)]

            slabs = {}   # (k, sj) -> tile

            def get_slab(k, sj):
                if (k, sj) not in slabs:
                    cn = min(G, int(Ck_tot[k]) - sj * G)  # chunks in this slab
                    t = slabp.tile([128, G, F], dt.float32, tag="slab")
                    it = idxp.tile([128, G * 8], dt.int16, tag="idx")
                    nc.sync.dma_start(
                        it[:, : cn * 8], idx_d[k][:, sj * G * 8: sj * G * 8 + cn * 8]
                    )
                    nc.gpsimd.dma_gather(
                        t[:, :cn, :],
                        x_src[k],
                        it[:, : cn * 8],
                        cn * CH,
                        cn * CH,
                        F,
                        # >64 descriptors in one packet wedges the SDMA engine
                        single_packet=False,
                    )
                    slabs[(k, sj)] = t
                return slabs[(k, sj)]

            pos = [0] * nk   # per-k consumed chunk count
            ci = 0           # global chunk index
            for b in range(nblk):
                pa = paggp.tile([128, 128], dt.float32, tag="pagg")
                nchunks = int(Cbk[b].sum())
                done = 0
                for k in range(nk):
                    for _ in range(int(Cbk[b, k])):
                        sj, off = divmod(pos[k], G)
                        slab = get_slab(k, sj)
                        s_t = sp.tile([128, 128], dt.float32, tag="s")
                        nc.vector.tensor_scalar(
                            s_t[:],
                            iota_sb[:],
                            rowloc_sb[:, ci: ci + 1],
                            normv_sb[:, ci: ci + 1],
                            mybir.AluOpType.is_equal,
                            mybir.AluOpType.mult,
                        )
                        nc.tensor.matmul(
                            pa[:],
                            slab[:, off, :],
                            s_t[:],
                            start=(done == 0),
                            stop=(done == nchunks - 1),
                        )
                        pos[k] += 1
                        ci += 1
                        done += 1
                # aggT [f, d] -> SBUF on scalar engine
                pre = prep.tile([128, 128], dt.float32, tag="pre")
                nc.scalar.activation(
                    pre[:], pa[:], mybir.ActivationFunctionType.Copy
                )
                po = poutp.tile([128, F], dt.float32, tag="pout")
                nc.tensor.matmul(po[:], pre[:], w_sb[:], start=True, stop=True)
                ob = obp.tile([128, F], dt.float32, tag="ob")
                nc.vector.tensor_add(ob[:], po[:], bias_sb[:])
                rows = min(128, npc - b * 128)
                nc.sync.dma_start(y_d[b * 128: b * 128 + rows, :], ob[:rows, :])

    nc.compile()
    return nc


def kernel(x, edge_index, weight, bias, _n_nodes=100000, _src_blk=32768, _g=16,
           _return_nc=False):
    x = np.ascontiguousarray(np.asarray(x, dtype=np.float32))
    edge_index = np.asarray(edge_index)
    weight = np.ascontiguousarray(np.asarray(weight, dtype=np.float32))
    bias = np.asarray(bias, dtype=np.float32)
    n_nodes = x.shape[0]
    assert n_nodes == _n_nodes and n_nodes % N_CORES == 0

    per_core, Cbk, Ck_tot, C_tot, npc, nblk, nk = _prep(
        x, edge_index, n_nodes, _src_blk
    )
    nc = _build(n_nodes, _src_blk, Cbk, Ck_tot, C_tot, npc, nblk, nk, _g)

    iota = np.tile(np.arange(128, dtype=np.float32), (128, 1))
    biasrep = np.tile(bias[None, :], (128, 1)).astype(np.float32)
    in_maps = [
        {
            "x": x,
            "w": weight,
            "iota": iota,
            "biasrep": biasrep,
            **per_core[ci],
        }
        for ci in range(N_CORES)
    ]
    res = run_bass_kernel_spmd(nc, in_maps, core_ids=list(range(N_CORES)))
    out = np.concatenate([res.results[ci]["y"] for ci in range(N_CORES)], axis=0)
    if _return_nc:
        return out, nc, in_maps
    return out


# revision 12
# speedup vs baseline: 14333.5495x; 14333.5495x over previous
"""GCN layer (gather-scale-scatter + dense transform) on 8 trn2 NeuronCores.

out[r] = (sum_{e:row[e]=r} norm_e * x[col_e]  (incl self loop norm=dis^2)) @ W + bias
with norm_e = dis[row]*dis[col], dis = rsqrt(1 + outdeg).

Sharding: destination nodes partitioned across 8 cores (12500 each); each
core is fully independent (x is replicated input; no collectives).

Device algorithm per core:
  - edges grouped host-side by (dest block of 128, src block of 32768),
    padded to chunks of 128 edges, chunk counts equalized across cores so a
    single NEFF serves all 8 cores.
  - dma_gather pulls x[col] rows (512B) from HBM into SBUF slabs.
  - per chunk, DVE builds S[e,d] = (iota[d]==rowloc[e]) * norm[e] in one
    tensor_scalar op; PE accumulates aggT[f,d] += xg.T @ S in PSUM.
  - per dest block: aggT -> SBUF, out = aggT.T @ W + bias -> HBM.
"""

import numpy as np

import concourse.bass as bass
import concourse.mybir as mybir
import concourse.tile as tile
from concourse import bacc
from concourse.bass_utils import run_bass_kernel_spmd

F = 128          # feature dim (in == out)
CH = 128         # edges per chunk
N_CORES = 8


def _prep(x, edge_index, n_nodes, src_blk):
    """Host-side integer/index preprocessing. Returns per-core device arrays
    and the static chunk-structure shared by all cores."""
    npc = n_nodes // N_CORES            # nodes per core
    nblk = (npc + 127) // 128           # dest blocks per core
    nk = (n_nodes + src_blk - 1) // src_blk  # src blocks

    r = np.asarray(edge_index[0], dtype=np.int64)
    c = np.asarray(edge_index[1], dtype=np.int64)
    deg = (np.bincount(r, minlength=n_nodes) + 1).astype(np.float64)
    dis = (1.0 / np.sqrt(deg)).astype(np.float32)

    loops = np.arange(n_nodes, dtype=np.int64)
    rr = np.concatenate([r, loops])
    cc = np.concatenate([c, loops])
    norm = dis[rr] * dis[cc]

    core = rr // npc
    rloc = rr - core * npc
    b_arr = rloc >> 7                   # dest block
    d_arr = (rloc & 127).astype(np.float32)   # local dest within block
    k_arr = cc // src_blk

    # group counts per (core, b, k) -> equalized chunk counts Cbk [nblk, nk]
    gid = (core * nblk + b_arr) * nk + k_arr
    counts = np.bincount(gid, minlength=N_CORES * nblk * nk).reshape(
        N_CORES, nblk, nk
    )
    Cbk = np.ceil(counts / CH).astype(np.int64).max(axis=0)  # [nblk, nk]
    Ck_tot = Cbk.sum(axis=0)            # [nk] chunks per src block
    C_tot = int(Cbk.sum())

    # chunk base offsets
    chunk_base = np.zeros((nblk, nk), np.int64)   # in global (b-major) order
    chunk_base.reshape(-1)[1:] = np.cumsum(Cbk.reshape(-1))[:-1]
    kpos_base = np.zeros((nblk, nk), np.int64)    # per-k (b-major within k)
    kpos_base[1:, :] = np.cumsum(Cbk, axis=0)[:-1, :]

    per_core = []
    for ci in range(N_CORES):
        sel = core == ci
        bs, ks, ds, cs, ns = b_arr[sel], k_arr[sel], d_arr[sel], cc[sel], norm[sel]
        order = np.lexsort((cs, ks, bs))
        bs, ks, ds, cs, ns = bs[order], ks[order], ds[order], cs[order], ns[order]
        # position within (b,k) group
        g = bs * nk + ks
        cnt = np.bincount(g, minlength=nblk * nk)
        gstart = np.zeros(nblk * nk, np.int64)
        gstart[1:] = np.cumsum(cnt)[:-1]
        j = np.arange(len(g)) - gstart[g]

        rowloc = np.full(C_tot * CH, -1.0, np.float32)
        normv = np.zeros(C_tot * CH, np.float32)
        slot = chunk_base[bs, ks] * CH + j
        rowloc[slot] = ds
        normv[slot] = ns

        idx_ks = []
        for k in range(nk):
            arr = np.zeros(int(Ck_tot[k]) * CH, np.int16)
            m = ks == k
            kslot = kpos_base[bs[m], k] * CH + j[m]
            arr[kslot] = (cs[m] - k * src_blk).astype(np.int16)
            # wrap: idx i -> [i % 16, i // 16], replicated to 128 partitions
            wrapped = arr.reshape(-1, 16).T          # [16, Ck*8]
            idx_ks.append(np.tile(wrapped, (8, 1)).copy())  # [128, Ck*8]

        per_core.append(
            {
                "rowloc": rowloc.reshape(C_tot, CH).T.copy(),  # [128, C_tot]
                "normv": normv.reshape(C_tot, CH).T.copy(),
                **{f"idx{k}": idx_ks[k] for k in range(nk)},
            }
        )

    return per_core, Cbk, Ck_tot, C_tot, npc, nblk, nk


def _build(n_nodes, src_blk, Cbk, Ck_tot, C_tot, npc, nblk, nk, G, wide=False):
    """Build the Bass program (shared by all cores).

    wide=True: accumulate two dest blocks per PSUM tile ([128, 256]) and run
    the segment matmuls in float32r, which streams at 1 cyc/row when the
    moving dim is >=256 (vs 4 for fp32) -- 2x PE time per chunk saved.
    """
    nc = bacc.Bacc(None, target_bir_lowering=False)
    dt = mybir.dt
    DW = 256 if wide else 128   # psum/S width in dest columns

    mm_t = dt.float32r if wide else dt.float32
    x_d = nc.dram_tensor("x", [n_nodes, F], mm_t, kind="ExternalInput")
    w_d = nc.dram_tensor("w", [F, F], dt.float32, kind="ExternalInput")
    iota_d = nc.dram_tensor("iota", [128, DW], dt.float32, kind="ExternalInput")
    bias_d = nc.dram_tensor("biasrep", [128, F], dt.float32, kind="ExternalInput")
    rowloc_d = nc.dram_tensor("rowloc", [128, C_tot], dt.float32, kind="ExternalInput")
    normv_d = nc.dram_tensor("normv", [128, C_tot], dt.float32, kind="ExternalInput")
    idx_d = [
        nc.dram_tensor(f"idx{k}", [128, int(Ck_tot[k]) * 8], dt.int16,
                       kind="ExternalInput")
        for k in range(nk)
    ]
    y_d = nc.dram_tensor("y", [npc, F], dt.float32, kind="ExternalOutput")

    with tile.TileContext(nc) as tc:
        with (
            tc.tile_pool(name="const", bufs=1) as constp,
            tc.tile_pool(name="slab", bufs=10) as slabp,
            tc.tile_pool(name="idxp", bufs=10) as idxp,
            tc.tile_pool(name="sp", bufs=8) as sp,
            tc.tile_pool(name="pre", bufs=3) as prep,
            tc.tile_pool(name="ob", bufs=3) as obp,
            tc.tile_pool(name="pagg", bufs=4, space="PSUM") as paggp,
            tc.tile_pool(name="pout", bufs=2, space="PSUM") as poutp,
        ):
            w_sb = constp.tile([F, F], dt.float32, tag="w")
            nc.sync.dma_start(w_sb[:], w_d[:])
            iota_sb = constp.tile([128, DW], dt.float32, tag="iota")
            nc.sync.dma_start(iota_sb[:], iota_d[:])
            bias_sb = constp.tile([128, F], dt.float32, tag="bias")
            nc.sync.dma_start(bias_sb[:], bias_d[:])
            rowloc_sb = constp.tile([128, C_tot], dt.float32, tag="rowloc")
            nc.sync.dma_start(rowloc_sb[:], rowloc_d[:])
            normv_sb = constp.tile([128, C_tot], dt.float32, tag="normv")
            nc.sync.dma_start(normv_sb[:], normv_d[:])

            x_src = [x_d[k * src_blk: min((k + 1) * src_blk, n_nodes), :]
                     for k in range(nk)]

            slabs = {}   # (k, sj) -> tile

            def get_slab(k, sj):
                if (k, sj) not in slabs:
                    cn = min(G, int(Ck_tot[k]) - sj * G)  # chunks in this slab
                    t = slabp.tile([128, G, F], mm_t, tag="slab")
                    it = idxp.tile([128, G * 8], dt.int16, tag="idx")
                    nc.sync.dma_start(
                        it[:, : cn * 8], idx_d[k][:, sj * G * 8: sj * G * 8 + cn * 8]
                    )
                    nc.gpsimd.dma_gather(
                        t[:, :cn, :],
                        x_src[k],
                        it[:, : cn * 8],
                        cn * CH,
                        cn * CH,
                        F,
                        # >64 descriptors in one packet wedges the SDMA engine
                        single_packet=False,
                    )
                    slabs[(k, sj)] = t
                return slabs[(k, sj)]

            BPG = DW // 128          # dest blocks per psum group
            S_BUFS = 8
            if wide:
                # Pre-zero every slot of each half-tag S pool. Chunks of an
                # even/odd block only ever write their own half of an
                # "s0"/"s1" tile, so the other half stays zero across slot
                # generations and the [128, DW] matmul read is safe.
                for h in range(BPG):
                    for _ in range(S_BUFS):
                        t0 = sp.tile([128, DW], mm_t, tag=f"s{h}")
                        nc.vector.memset(t0[:].bitcast(dt.float32), 0.0)

            pos = [0] * nk   # per-k consumed chunk count
            ci = 0           # global chunk index
            for g in range((nblk + BPG - 1) // BPG):
                blocks = [b for b in range(g * BPG, min((g + 1) * BPG, nblk))]
                pa = paggp.tile([128, DW], dt.float32, tag="pagg")
                nchunks = int(sum(Cbk[b].sum() for b in blocks))
                done = 0
                for b in blocks:
                    h = b - g * BPG
                    hs = h * 128
                    for k in range(nk):
                        for _ in range(int(Cbk[b, k])):
                            sj, off = divmod(pos[k], G)
                            slab = get_slab(k, sj)
                            s_t = sp.tile([128, DW], mm_t, tag=f"s{h}")
                            nc.vector.tensor_scalar(
                                s_t[:, hs: hs + 128],
                                iota_sb[:, :128],
                                rowloc_sb[:, ci: ci + 1],
                                normv_sb[:, ci: ci + 1],
                                mybir.AluOpType.is_equal,
                                mybir.AluOpType.mult,
                            )
                            nc.tensor.matmul(
                                pa[:],
                                slab[:, off, :],
                                s_t[:],
                                start=(done == 0),
                                stop=(done == nchunks - 1),
                            )
                            pos[k] += 1
                            ci += 1
                            done += 1
                # aggT [f, d-group] -> SBUF on scalar engine
                pre = prep.tile([128, DW], dt.float32, tag="pre")
                nc.scalar.activation(
                    pre[:], pa[:], mybir.ActivationFunctionType.Copy
                )
                for b in blocks:
                    h = b - g * BPG
                    hs = h * 128
                    po = poutp.tile([128, F], dt.float32, tag="pout")
                    nc.tensor.matmul(
                        po[:], pre[:, hs: hs + 128], w_sb[:], start=True, stop=True
                    )
                    ob = obp.tile([128, F], dt.float32, tag="ob")
                    nc.vector.tensor_add(ob[:], po[:], bias_sb[:])
                    rows = min(128, npc - b * 128)
                    nc.sync.dma_start(
                        y_d[b * 128: b * 128 + rows, :], ob[:rows, :]
                    )

    nc.compile()
    return nc


def kernel(x, edge_index, weight, bias, _n_nodes=100000, _src_blk=32768, _g=16,
           _wide=False, _return_nc=False):
    x = np.ascontiguousarray(np.asarray(x, dtype=np.float32))
    edge_index = np.asarray(edge_index)
    weight = np.ascontiguousarray(np.asarray(weight, dtype=np.float32))
    bias = np.asarray(bias, dtype=np.float32)
    n_nodes = x.shape[0]
    assert n_nodes == _n_nodes and n_nodes % N_CORES == 0

    per_core, Cbk, Ck_tot, C_tot, npc, nblk, nk = _prep(
        x, edge_index, n_nodes, _src_blk
    )
    nc = _build(n_nodes, _src_blk, Cbk, Ck_tot, C_tot, npc, nblk, nk, _g,
                wide=_wide)

    iota = np.tile(np.arange(256 if _wide else 128, dtype=np.float32), (128, 1))
    biasrep = np.tile(bias[None, :], (128, 1)).astype(np.float32)
    in_maps = [
        {
            "x": x,
            "w": weight,
            "iota": iota,
            "biasrep": biasrep,
            **per_core[ci],
        }
        for ci in range(N_CORES)
    ]
    res = run_bass_kernel_spmd(nc, in_maps, core_ids=list(range(N_CORES)))
    out = np.concatenate([res.results[ci]["y"] for ci in range(N_CORES)], axis=0)
    if _return_nc:
        return out, nc, in_maps
    return out


# revision 15
# speedup vs baseline: 15345.9869x; 1.0706x over previous
"""GCN layer (gather-scale-scatter + dense transform) on 8 trn2 NeuronCores.

out[r] = (sum_{e:row[e]=r} norm_e * x[col_e]  (incl self loop norm=dis^2)) @ W + bias
with norm_e = dis[row]*dis[col], dis = rsqrt(1 + outdeg).

Sharding: destination nodes partitioned across 8 cores (12500 each); each
core is fully independent (x is replicated input; no collectives).

Device algorithm per core:
  - edges grouped host-side by (dest block of 104, src block of 32768),
    padded to chunks of 128 edges, chunk counts equalized across cores so a
    single NEFF serves all 8 cores.
  - dma_gather pulls x[col] rows (512B) from HBM into SBUF slabs.
  - per chunk, DVE builds S[e,d] = (iota[d]==rowloc[e]) * norm[e] in one
    tensor_scalar op; PE accumulates aggT[f,d] += xg.T @ S in PSUM.
  - per dest block: aggT -> SBUF, out = aggT.T @ W + bias -> HBM.
  Dest-block width 104 balances PE time (4 cyc/row fp32 matmul, prop.
  to width) against gather padding (prop. to group count).
"""

import numpy as np

import concourse.bass as bass
import concourse.mybir as mybir
import concourse.tile as tile
from concourse import bacc
from concourse.bass_utils import run_bass_kernel_spmd

F = 128          # feature dim (in == out)
CH = 128         # edges per chunk
N_CORES = 8


def _prep(x, edge_index, n_nodes, src_blk, dblk=128):
    """Host-side integer/index preprocessing. Returns per-core device arrays
    and the static chunk-structure shared by all cores."""
    npc = n_nodes // N_CORES            # nodes per core
    nblk = (npc + dblk - 1) // dblk     # dest blocks per core
    nk = (n_nodes + src_blk - 1) // src_blk  # src blocks

    r = np.asarray(edge_index[0], dtype=np.int64)
    c = np.asarray(edge_index[1], dtype=np.int64)
    deg = (np.bincount(r, minlength=n_nodes) + 1).astype(np.float64)
    dis = (1.0 / np.sqrt(deg)).astype(np.float32)

    loops = np.arange(n_nodes, dtype=np.int64)
    rr = np.concatenate([r, loops])
    cc = np.concatenate([c, loops])
    norm = dis[rr] * dis[cc]

    core = rr // npc
    rloc = rr - core * npc
    b_arr = rloc // dblk                # dest block
    d_arr = (rloc % dblk).astype(np.float32)  # local dest within block
    k_arr = cc // src_blk

    # group counts per (core, b, k) -> equalized chunk counts Cbk [nblk, nk]
    gid = (core * nblk + b_arr) * nk + k_arr
    counts = np.bincount(gid, minlength=N_CORES * nblk * nk).reshape(
        N_CORES, nblk, nk
    )
    Cbk = np.ceil(counts / CH).astype(np.int64).max(axis=0)  # [nblk, nk]
    Ck_tot = Cbk.sum(axis=0)            # [nk] chunks per src block
    C_tot = int(Cbk.sum())

    # chunk base offsets
    chunk_base = np.zeros((nblk, nk), np.int64)   # in global (b-major) order
    chunk_base.reshape(-1)[1:] = np.cumsum(Cbk.reshape(-1))[:-1]
    kpos_base = np.zeros((nblk, nk), np.int64)    # per-k (b-major within k)
    kpos_base[1:, :] = np.cumsum(Cbk, axis=0)[:-1, :]

    per_core = []
    for ci in range(N_CORES):
        sel = core == ci
        bs, ks, ds, cs, ns = b_arr[sel], k_arr[sel], d_arr[sel], cc[sel], norm[sel]
        order = np.lexsort((cs, ks, bs))
        bs, ks, ds, cs, ns = bs[order], ks[order], ds[order], cs[order], ns[order]
        # position within (b,k) group
        g = bs * nk + ks
        cnt = np.bincount(g, minlength=nblk * nk)
        gstart = np.zeros(nblk * nk, np.int64)
        gstart[1:] = np.cumsum(cnt)[:-1]
        j = np.arange(len(g)) - gstart[g]

        rowloc = np.full(C_tot * CH, -1.0, np.float32)
        normv = np.zeros(C_tot * CH, np.float32)
        slot = chunk_base[bs, ks] * CH + j
        rowloc[slot] = ds
        normv[slot] = ns

        idx_ks = []
        for k in range(nk):
            arr = np.zeros(int(Ck_tot[k]) * CH, np.int16)
            m = ks == k
            kslot = kpos_base[bs[m], k] * CH + j[m]
            arr[kslot] = (cs[m] - k * src_blk).astype(np.int16)
            # wrap: idx i -> [i % 16, i // 16], replicated to 128 partitions
            wrapped = arr.reshape(-1, 16).T          # [16, Ck*8]
            idx_ks.append(np.tile(wrapped, (8, 1)).copy())  # [128, Ck*8]

        per_core.append(
            {
                "rowloc": rowloc.reshape(C_tot, CH).T.copy(),  # [128, C_tot]
                "normv": normv.reshape(C_tot, CH).T.copy(),
                **{f"idx{k}": idx_ks[k] for k in range(nk)},
            }
        )

    return per_core, Cbk, Ck_tot, C_tot, npc, nblk, nk


def _build(n_nodes, src_blk, Cbk, Ck_tot, C_tot, npc, nblk, nk, G, wide=False,
           dblk=128):
    """Build the Bass program (shared by all cores).

    wide=True: accumulate two dest blocks per PSUM tile ([128, 256]) and run
    the segment matmuls in float32r, which streams at 1 cyc/row when the
    moving dim is >=256 (vs 4 for fp32) -- 2x PE time per chunk saved.
    """
    nc = bacc.Bacc(None, target_bir_lowering=False)
    dt = mybir.dt
    assert dblk == 128 or not wide
    DW = 256 if wide else dblk  # psum/S width in dest columns

    mm_t = dt.float32r if wide else dt.float32
    x_d = nc.dram_tensor("x", [n_nodes, F], mm_t, kind="ExternalInput")
    w_d = nc.dram_tensor("w", [F, F], dt.float32, kind="ExternalInput")
    iota_d = nc.dram_tensor("iota", [128, DW], dt.float32, kind="ExternalInput")
    bias_d = nc.dram_tensor("biasrep", [128, F], dt.float32, kind="ExternalInput")
    rowloc_d = nc.dram_tensor("rowloc", [128, C_tot], dt.float32, kind="ExternalInput")
    normv_d = nc.dram_tensor("normv", [128, C_tot], dt.float32, kind="ExternalInput")
    idx_d = [
        nc.dram_tensor(f"idx{k}", [128, int(Ck_tot[k]) * 8], dt.int16,
                       kind="ExternalInput")
        for k in range(nk)
    ]
    y_d = nc.dram_tensor("y", [npc, F], dt.float32, kind="ExternalOutput")

    with tile.TileContext(nc) as tc:
        with (
            tc.tile_pool(name="const", bufs=1) as constp,
            tc.tile_pool(name="slab", bufs=10) as slabp,
            tc.tile_pool(name="idxp", bufs=10) as idxp,
            tc.tile_pool(name="sp", bufs=8) as sp,
            tc.tile_pool(name="pre", bufs=3) as prep,
            tc.tile_pool(name="ob", bufs=3) as obp,
            tc.tile_pool(name="pagg", bufs=4, space="PSUM") as paggp,
            tc.tile_pool(name="pout", bufs=2, space="PSUM") as poutp,
        ):
            w_sb = constp.tile([F, F], dt.float32, tag="w")
            nc.sync.dma_start(w_sb[:], w_d[:])
            iota_sb = constp.tile([128, DW], dt.float32, tag="iota")
            nc.sync.dma_start(iota_sb[:], iota_d[:])
            bias_sb = constp.tile([128, F], dt.float32, tag="bias")
            nc.sync.dma_start(bias_sb[:], bias_d[:])
            rowloc_sb = constp.tile([128, C_tot], dt.float32, tag="rowloc")
            nc.sync.dma_start(rowloc_sb[:], rowloc_d[:])
            normv_sb = constp.tile([128, C_tot], dt.float32, tag="normv")
            nc.sync.dma_start(normv_sb[:], normv_d[:])

            x_src = [x_d[k * src_blk: min((k + 1) * src_blk, n_nodes), :]
                     for k in range(nk)]

            slabs = {}   # (k, sj) -> tile

            def get_slab(k, sj):
                if (k, sj) not in slabs:
                    cn = min(G, int(Ck_tot[k]) - sj * G)  # chunks in this slab
                    t = slabp.tile([128, G, F], mm_t, tag="slab")
                    it = idxp.tile([128, G * 8], dt.int16, tag="idx")
                    nc.sync.dma_start(
                        it[:, : cn * 8], idx_d[k][:, sj * G * 8: sj * G * 8 + cn * 8]
                    )
                    nc.gpsimd.dma_gather(
                        t[:, :cn, :],
                        x_src[k],
                        it[:, : cn * 8],
                        cn * CH,
                        cn * CH,
                        F,
                        # >64 descriptors in one packet wedges the SDMA engine
                        single_packet=False,
                    )
                    slabs[(k, sj)] = t
                return slabs[(k, sj)]

            BPG = 2 if wide else 1   # dest blocks per psum group
            S_BUFS = 8
            if wide:
                # Pre-zero every slot of each half-tag S pool. Chunks of an
                # even/odd block only ever write their own half of an
                # "s0"/"s1" tile, so the other half stays zero across slot
                # generations and the [128, DW] matmul read is safe.
                for h in range(BPG):
                    for _ in range(S_BUFS):
                        t0 = sp.tile([128, DW], mm_t, tag=f"s{h}")
                        nc.vector.memset(t0[:].bitcast(dt.float32), 0.0)

            pos = [0] * nk   # per-k consumed chunk count
            ci = 0           # global chunk index
            for g in range((nblk + BPG - 1) // BPG):
                blocks = [b for b in range(g * BPG, min((g + 1) * BPG, nblk))]
                pa = paggp.tile([128, DW], dt.float32, tag="pagg")
                nchunks = int(sum(Cbk[b].sum() for b in blocks))
                done = 0
                for b in blocks:
                    h = b - g * BPG
                    hs = h * (DW // BPG)
                    for k in range(nk):
                        for _ in range(int(Cbk[b, k])):
                            sj, off = divmod(pos[k], G)
                            slab = get_slab(k, sj)
                            s_t = sp.tile([128, DW], mm_t, tag=f"s{h}")
                            nc.vector.tensor_scalar(
                                s_t[:, hs: hs + dblk],
                                iota_sb[:, :dblk],
                                rowloc_sb[:, ci: ci + 1],
                                normv_sb[:, ci: ci + 1],
                                mybir.AluOpType.is_equal,
                                mybir.AluOpType.mult,
                            )
                            nc.tensor.matmul(
                                pa[:],
                                slab[:, off, :],
                                s_t[:],
                                start=(done == 0),
                                stop=(done == nchunks - 1),
                            )
                            pos[k] += 1
                            ci += 1
                            done += 1
                # aggT [f, d-group] -> SBUF on scalar engine
                pre = prep.tile([128, DW], dt.float32, tag="pre")
                nc.scalar.activation(
                    pre[:], pa[:], mybir.ActivationFunctionType.Copy
                )
                for b in blocks:
                    h = b - g * BPG
                    hs = h * (DW // BPG)
                    bw = DW // BPG
                    po = poutp.tile([128, F], dt.float32, tag="pout")
                    nc.tensor.matmul(
                        po[:bw, :], pre[:, hs: hs + bw], w_sb[:],
                        start=True, stop=True
                    )
                    ob = obp.tile([128, F], dt.float32, tag="ob")
                    nc.vector.tensor_add(ob[:bw, :], po[:bw, :], bias_sb[:bw, :])
                    rows = min(bw, npc - b * bw)
                    nc.sync.dma_start(
                        y_d[b * bw: b * bw + rows, :], ob[:rows, :]
                    )

    nc.compile()
    return nc


def kernel(x, edge_index, weight, bias, _n_nodes=100000, _src_blk=32768, _g=16,
           _wide=False, _dblk=104, _return_nc=False):
    x = np.ascontiguousarray(np.asarray(x, dtype=np.float32))
    edge_index = np.asarray(edge_index)
    weight = np.ascontiguousarray(np.asarray(weight, dtype=np.float32))
    bias = np.asarray(bias, dtype=np.float32)
    n_nodes = x.shape[0]
    assert n_nodes == _n_nodes and n_nodes % N_CORES == 0

    per_core, Cbk, Ck_tot, C_tot, npc, nblk, nk = _prep(
        x, edge_index, n_nodes, _src_blk, dblk=_dblk
    )
    nc = _build(n_nodes, _src_blk, Cbk, Ck_tot, C_tot, npc, nblk, nk, _g,
                wide=_wide, dblk=_dblk)

    iota = np.tile(np.arange(256 if _wide else _dblk, dtype=np.float32), (128, 1))
    biasrep = np.tile(bias[None, :], (128, 1)).astype(np.float32)
    in_maps = [
        {
            "x": x,
            "w": weight,
            "iota": iota,
            "biasrep": biasrep,
            **per_core[ci],
        }
        for ci in range(N_CORES)
    ]
    res = run_bass_kernel_spmd(nc, in_maps, core_ids=list(range(N_CORES)))
    out = np.concatenate([res.results[ci]["y"] for ci in range(N_CORES)], axis=0)
    if _return_nc:
        return out, nc, in_maps
    return out


# revision 18
# speedup vs baseline: 15482.8507x; 1.0089x over previous
"""GCN layer (gather-scale-scatter + dense transform) on 8 trn2 NeuronCores.

out[r] = (sum_{e:row[e]=r} norm_e * x[col_e]  (incl self loop norm=dis^2)) @ W + bias
with norm_e = dis[row]*dis[col], dis = rsqrt(1 + outdeg).

Sharding: destination nodes partitioned across 8 cores (12500 each); each
core is fully independent (x is replicated input; no collectives).

Device algorithm per core:
  - edges grouped host-side by (dest block of 104, src block of 32768),
    padded to chunks of 128 edges, chunk counts equalized across cores so a
    single NEFF serves all 8 cores.
  - dma_gather pulls x[col] rows (512B) from HBM into SBUF slabs.
  - per chunk, DVE builds S[e,d] = (iota[d]==rowloc[e]) * norm[e] in one
    tensor_scalar op; PE accumulates aggT[f,d] += xg.T @ S in PSUM.
  - per dest block: aggT -> SBUF, out = aggT.T @ W + bias -> HBM.
  Dest-block width 104 balances PE time (4 cyc/row fp32 matmul, prop.
  to width) against gather padding (prop. to group count).
"""

import numpy as np

import concourse.bass as bass
import concourse.mybir as mybir
import concourse.tile as tile
from concourse import bacc
from concourse.bass_utils import run_bass_kernel_spmd

F = 128          # feature dim (in == out)
CH = 128         # edges per chunk
N_CORES = 8


def _prep(x, edge_index, n_nodes, src_blk, dblk=128):
    """Host-side integer/index preprocessing. Returns per-core device arrays
    and the static chunk-structure shared by all cores."""
    npc = n_nodes // N_CORES            # nodes per core
    nblk = (npc + dblk - 1) // dblk     # dest blocks per core
    nk = (n_nodes + src_blk - 1) // src_blk  # src blocks

    r = np.asarray(edge_index[0], dtype=np.int64)
    c = np.asarray(edge_index[1], dtype=np.int64)
    deg = (np.bincount(r, minlength=n_nodes) + 1).astype(np.float64)
    dis = (1.0 / np.sqrt(deg)).astype(np.float32)

    loops = np.arange(n_nodes, dtype=np.int64)
    rr = np.concatenate([r, loops])
    cc = np.concatenate([c, loops])
    norm = dis[rr] * dis[cc]

    core = rr // npc
    rloc = rr - core * npc
    b_arr = rloc // dblk                # dest block
    d_arr = (rloc % dblk).astype(np.float32)  # local dest within block
    k_arr = cc // src_blk

    # group counts per (core, b, k) -> equalized chunk counts Cbk [nblk, nk]
    gid = (core * nblk + b_arr) * nk + k_arr
    counts = np.bincount(gid, minlength=N_CORES * nblk * nk).reshape(
        N_CORES, nblk, nk
    )
    Cbk = np.ceil(counts / CH).astype(np.int64).max(axis=0)  # [nblk, nk]
    maxcnt = counts.max(axis=0)         # [nblk, nk] max real edges per group
    Ck_tot = Cbk.sum(axis=0)            # [nk] chunks per src block
    C_tot = int(Cbk.sum())

    # chunk base offsets
    chunk_base = np.zeros((nblk, nk), np.int64)   # in global (b-major) order
    chunk_base.reshape(-1)[1:] = np.cumsum(Cbk.reshape(-1))[:-1]
    kpos_base = np.zeros((nblk, nk), np.int64)    # per-k (b-major within k)
    kpos_base[1:, :] = np.cumsum(Cbk, axis=0)[:-1, :]

    per_core = []
    for ci in range(N_CORES):
        sel = core == ci
        bs, ks, ds, cs, ns = b_arr[sel], k_arr[sel], d_arr[sel], cc[sel], norm[sel]
        order = np.lexsort((cs, ks, bs))
        bs, ks, ds, cs, ns = bs[order], ks[order], ds[order], cs[order], ns[order]
        # position within (b,k) group
        g = bs * nk + ks
        cnt = np.bincount(g, minlength=nblk * nk)
        gstart = np.zeros(nblk * nk, np.int64)
        gstart[1:] = np.cumsum(cnt)[:-1]
        j = np.arange(len(g)) - gstart[g]

        rowloc = np.full(C_tot * CH, -1.0, np.float32)
        normv = np.zeros(C_tot * CH, np.float32)
        slot = chunk_base[bs, ks] * CH + j
        rowloc[slot] = ds
        normv[slot] = ns

        idx_ks = []
        for k in range(nk):
            arr = np.zeros(int(Ck_tot[k]) * CH, np.int16)
            # -1 tail beyond the cross-core max count: dma_gather skips the
            # transfer for trailing negative idxs (slot keeps stale data,
            # cancelled by rowloc=-1 in the segment matrix)
            maxc_rep = np.repeat(maxcnt[:, k], Cbk[:, k] * CH)
            base_rep = np.repeat(kpos_base[:, k] * CH, Cbk[:, k] * CH)
            posrel = np.arange(int(Ck_tot[k]) * CH) - base_rep
            arr[posrel >= maxc_rep] = -1
            m = ks == k
            kslot = kpos_base[bs[m], k] * CH + j[m]
            arr[kslot] = (cs[m] - k * src_blk).astype(np.int16)
            # wrap: idx i -> [i % 16, i // 16], replicated to 128 partitions
            wrapped = arr.reshape(-1, 16).T          # [16, Ck*8]
            idx_ks.append(np.tile(wrapped, (8, 1)).copy())  # [128, Ck*8]

        per_core.append(
            {
                "rowloc": rowloc.reshape(C_tot, CH).T.copy(),  # [128, C_tot]
                "normv": normv.reshape(C_tot, CH).T.copy(),
                **{f"idx{k}": idx_ks[k] for k in range(nk)},
            }
        )

    return per_core, Cbk, Ck_tot, C_tot, npc, nblk, nk, maxcnt


def _build(n_nodes, src_blk, Cbk, Ck_tot, C_tot, npc, nblk, nk, G, wide=False,
           dblk=128, maxcnt=None):
    """Build the Bass program (shared by all cores).

    wide=True: accumulate two dest blocks per PSUM tile ([128, 256]) and run
    the segment matmuls in float32r, which streams at 1 cyc/row when the
    moving dim is >=256 (vs 4 for fp32) -- 2x PE time per chunk saved.
    """
    nc = bacc.Bacc(None, target_bir_lowering=False)
    dt = mybir.dt
    assert dblk == 128 or not wide
    DW = 256 if wide else dblk  # psum/S width in dest columns

    mm_t = dt.float32r if wide else dt.float32
    x_d = nc.dram_tensor("x", [n_nodes, F], mm_t, kind="ExternalInput")
    w_d = nc.dram_tensor("w", [F, F], dt.float32, kind="ExternalInput")
    iota_d = nc.dram_tensor("iota", [128, DW], dt.float32, kind="ExternalInput")
    bias_d = nc.dram_tensor("biasrep", [128, F], dt.float32, kind="ExternalInput")
    rowloc_d = nc.dram_tensor("rowloc", [128, C_tot], dt.float32, kind="ExternalInput")
    normv_d = nc.dram_tensor("normv", [128, C_tot], dt.float32, kind="ExternalInput")
    idx_d = [
        nc.dram_tensor(f"idx{k}", [128, int(Ck_tot[k]) * 8], dt.int16,
                       kind="ExternalInput")
        for k in range(nk)
    ]
    y_d = nc.dram_tensor("y", [npc, F], dt.float32, kind="ExternalOutput")

    with tile.TileContext(nc) as tc:
        with (
            tc.tile_pool(name="const", bufs=1) as constp,
            tc.tile_pool(name="slab", bufs=10) as slabp,
            tc.tile_pool(name="idxp", bufs=10) as idxp,
            tc.tile_pool(name="sp", bufs=8) as sp,
            tc.tile_pool(name="pre", bufs=3) as prep,
            tc.tile_pool(name="ob", bufs=3) as obp,
            tc.tile_pool(name="pagg", bufs=4, space="PSUM") as paggp,
            tc.tile_pool(name="pout", bufs=2, space="PSUM") as poutp,
        ):
            w_sb = constp.tile([F, F], dt.float32, tag="w")
            nc.sync.dma_start(w_sb[:], w_d[:])
            iota_sb = constp.tile([128, DW], dt.float32, tag="iota")
            nc.sync.dma_start(iota_sb[:], iota_d[:])
            bias_sb = constp.tile([128, F], dt.float32, tag="bias")
            nc.sync.dma_start(bias_sb[:], bias_d[:])
            rowloc_sb = constp.tile([128, C_tot], dt.float32, tag="rowloc")
            nc.sync.dma_start(rowloc_sb[:], rowloc_d[:])
            normv_sb = constp.tile([128, C_tot], dt.float32, tag="normv")
            nc.sync.dma_start(normv_sb[:], normv_d[:])

            x_src = [x_d[k * src_blk: min((k + 1) * src_blk, n_nodes), :]
                     for k in range(nk)]

            # Pre-zero all slab slots: trailing -1 gather idxs skip the
            # transfer, so skipped slots read stale slot data; generation-0
            # slots would otherwise be uninitialized (NaN*0=NaN in PSUM).
            maxcbk = int(Cbk.max())
            for _ in range(10):
                t0 = slabp.tile([128, maxcbk, F], mm_t, tag="slab")
                z_ap = t0[:] if mm_t == dt.float32 else t0[:].bitcast(dt.float32)
                nc.vector.memset(z_ap, 0.0)

            BPG = 2 if wide else 1   # dest blocks per psum group
            S_BUFS = 8
            if wide:
                # Pre-zero every slot of each half-tag S pool. Chunks of an
                # even/odd block only ever write their own half of an
                # "s0"/"s1" tile, so the other half stays zero across slot
                # generations and the [128, DW] matmul read is safe.
                for h in range(BPG):
                    for _ in range(S_BUFS):
                        t0 = sp.tile([128, DW], mm_t, tag=f"s{h}")
                        nc.vector.memset(t0[:].bitcast(dt.float32), 0.0)

            pos = [0] * nk   # per-k consumed chunk count
            ci = 0           # global chunk index
            for g in range((nblk + BPG - 1) // BPG):
                blocks = [b for b in range(g * BPG, min((g + 1) * BPG, nblk))]
                pa = paggp.tile([128, DW], dt.float32, tag="pagg")
                nchunks = int(sum(Cbk[b].sum() for b in blocks))
                done = 0
                for b in blocks:
                    h = b - g * BPG
                    hs = h * (DW // BPG)
                    for k in range(nk):
                        cbk = int(Cbk[b, k])
                        if cbk == 0:
                            continue
                        # one gather per (b,k) group: trailing -1 idxs (the
                        # chunk-rounding pad beyond the max-core edge count)
                        # transfer nothing; num_idxs_reg = that max count,
                        # identical on every core by construction.
                        slab = slabp.tile([128, cbk, F], mm_t, tag="slab")
                        it = idxp.tile([128, cbk * 8], dt.int16, tag="idx")
                        base = pos[k]
                        nc.sync.dma_start(
                            it[:], idx_d[k][:, base * 8: (base + cbk) * 8]
                        )
                        nc.gpsimd.dma_gather(
                            slab[:],
                            x_src[k],
                            it[:],
                            cbk * CH,
                            int(maxcnt[b, k]),
                            F,
                            # >64 descs in one packet wedges the SDMA engine
                            single_packet=False,
                        )
                        pos[k] += cbk
                        for off in range(cbk):
                            s_t = sp.tile([128, DW], mm_t, tag=f"s{h}")
                            nc.vector.tensor_scalar(
                                s_t[:, hs: hs + dblk],
                                iota_sb[:, :dblk],
                                rowloc_sb[:, ci: ci + 1],
                                normv_sb[:, ci: ci + 1],
                                mybir.AluOpType.is_equal,
                                mybir.AluOpType.mult,
                            )
                            nc.tensor.matmul(
                                pa[:],
                                slab[:, off, :],
                                s_t[:],
                                start=(done == 0),
                                stop=(done == nchunks - 1),
                            )
                            ci += 1
                            done += 1
                # aggT [f, d-group] -> SBUF on scalar engine
                pre = prep.tile([128, DW], dt.float32, tag="pre")
                nc.scalar.activation(
                    pre[:], pa[:], mybir.ActivationFunctionType.Copy
                )
                for b in blocks:
                    h = b - g * BPG
                    hs = h * (DW // BPG)
                    bw = DW // BPG
                    po = poutp.tile([128, F], dt.float32, tag="pout")
                    nc.tensor.matmul(
                        po[:bw, :], pre[:, hs: hs + bw], w_sb[:],
                        start=True, stop=True
                    )
                    ob = obp.tile([128, F], dt.float32, tag="ob")
                    nc.vector.tensor_add(ob[:bw, :], po[:bw, :], bias_sb[:bw, :])
                    rows = min(bw, npc - b * bw)
                    nc.sync.dma_start(
                        y_d[b * bw: b * bw + rows, :], ob[:rows, :]
                    )

    nc.compile()
    return nc


def kernel(x, edge_index, weight, bias, _n_nodes=100000, _src_blk=32768, _g=16,
           _wide=False, _dblk=104, _return_nc=False):
    x = np.ascontiguousarray(np.asarray(x, dtype=np.float32))
    edge_index = np.asarray(edge_index)
    weight = np.ascontiguousarray(np.asarray(weight, dtype=np.float32))
    bias = np.asarray(bias, dtype=np.float32)
    n_nodes = x.shape[0]
    assert n_nodes == _n_nodes and n_nodes % N_CORES == 0

    per_core, Cbk, Ck_tot, C_tot, npc, nblk, nk, maxcnt = _prep(
        x, edge_index, n_nodes, _src_blk, dblk=_dblk
    )
    nc = _build(n_nodes, _src_blk, Cbk, Ck_tot, C_tot, npc, nblk, nk, _g,
                wide=_wide, dblk=_dblk, maxcnt=maxcnt)

    iota = np.tile(np.arange(256 if _wide else _dblk, dtype=np.float32), (128, 1))
    biasrep = np.tile(bias[None, :], (128, 1)).astype(np.float32)
    in_maps = [
        {
            "x": x,
            "w": weight,
            "iota": iota,
            "biasrep": biasrep,
            **per_core[ci],
        }
        for ci in range(N_CORES)
    ]
    res = run_bass_kernel_spmd(nc, in_maps, core_ids=list(range(N_CORES)))
    out = np.concatenate([res.results[ci]["y"] for ci in range(N_CORES)], axis=0)
    if _return_nc:
        return out, nc, in_maps
    return out


# revision 19
# speedup vs baseline: 15568.5897x; 1.0055x over previous
"""GCN layer (gather-scale-scatter + dense transform) on 8 trn2 NeuronCores.

out[r] = (sum_{e:row[e]=r} norm_e * x[col_e]  (incl self loop norm=dis^2)) @ W + bias
with norm_e = dis[row]*dis[col], dis = rsqrt(1 + outdeg).

Sharding: destination nodes partitioned across 8 cores (12500 each); each
core is fully independent (x is replicated input; no collectives).

Device algorithm per core:
  - edges grouped host-side by (dest block of 104, src block of 32768),
    padded to chunks of 128 edges, chunk counts equalized across cores so a
    single NEFF serves all 8 cores.
  - dma_gather pulls x[col] rows (512B) from HBM into SBUF slabs.
  - per chunk, DVE builds S[e,d] = (iota[d]==rowloc[e]) * norm[e] in one
    tensor_scalar op; PE accumulates aggT[f,d] += xg.T @ S in PSUM.
  - per dest block: aggT -> SBUF, out = aggT.T @ W + bias -> HBM.
  Dest-block width 104 balances PE time (4 cyc/row fp32 matmul, prop.
  to width) against gather padding (prop. to group count).
"""

import numpy as np

import concourse.bass as bass
import concourse.mybir as mybir
import concourse.tile as tile
from concourse import bacc
from concourse.bass_utils import run_bass_kernel_spmd

F = 128          # feature dim (in == out)
CH = 128         # edges per chunk
N_CORES = 8


def _prep(x, edge_index, n_nodes, src_blk, dblk=128):
    """Host-side integer/index preprocessing. Returns per-core device arrays
    and the static chunk-structure shared by all cores."""
    npc = n_nodes // N_CORES            # nodes per core
    nblk = (npc + dblk - 1) // dblk     # dest blocks per core
    nk = (n_nodes + src_blk - 1) // src_blk  # src blocks

    r = np.asarray(edge_index[0], dtype=np.int64)
    c = np.asarray(edge_index[1], dtype=np.int64)
    deg = (np.bincount(r, minlength=n_nodes) + 1).astype(np.float64)
    dis = (1.0 / np.sqrt(deg)).astype(np.float32)

    loops = np.arange(n_nodes, dtype=np.int64)
    rr = np.concatenate([r, loops])
    cc = np.concatenate([c, loops])
    norm = dis[rr] * dis[cc]

    core = rr // npc
    rloc = rr - core * npc
    b_arr = rloc // dblk                # dest block
    d_arr = (rloc % dblk).astype(np.float32)  # local dest within block
    k_arr = cc // src_blk

    # group counts per (core, b, k) -> equalized chunk counts Cbk [nblk, nk]
    gid = (core * nblk + b_arr) * nk + k_arr
    counts = np.bincount(gid, minlength=N_CORES * nblk * nk).reshape(
        N_CORES, nblk, nk
    )
    Cbk = np.ceil(counts / CH).astype(np.int64).max(axis=0)  # [nblk, nk]
    maxcnt = counts.max(axis=0)         # [nblk, nk] max real edges per group
    Ck_tot = Cbk.sum(axis=0)            # [nk] chunks per src block
    C_tot = int(Cbk.sum())

    # chunk base offsets
    chunk_base = np.zeros((nblk, nk), np.int64)   # in global (b-major) order
    chunk_base.reshape(-1)[1:] = np.cumsum(Cbk.reshape(-1))[:-1]
    kpos_base = np.zeros((nblk, nk), np.int64)    # per-k (b-major within k)
    kpos_base[1:, :] = np.cumsum(Cbk, axis=0)[:-1, :]

    per_core = []
    for ci in range(N_CORES):
        sel = core == ci
        bs, ks, ds, cs, ns = b_arr[sel], k_arr[sel], d_arr[sel], cc[sel], norm[sel]
        order = np.lexsort((cs, ks, bs))
        bs, ks, ds, cs, ns = bs[order], ks[order], ds[order], cs[order], ns[order]
        # position within (b,k) group
        g = bs * nk + ks
        cnt = np.bincount(g, minlength=nblk * nk)
        gstart = np.zeros(nblk * nk, np.int64)
        gstart[1:] = np.cumsum(cnt)[:-1]
        j = np.arange(len(g)) - gstart[g]

        rowloc = np.full(C_tot * CH, -1.0, np.float32)
        normv = np.zeros(C_tot * CH, np.float32)
        slot = chunk_base[bs, ks] * CH + j
        rowloc[slot] = ds
        normv[slot] = ns

        idx_ks = []
        for k in range(nk):
            arr = np.zeros(int(Ck_tot[k]) * CH, np.int16)
            # -1 tail beyond the cross-core max count: dma_gather skips the
            # transfer for trailing negative idxs (slot keeps stale data,
            # cancelled by rowloc=-1 in the segment matrix)
            maxc_rep = np.repeat(maxcnt[:, k], Cbk[:, k] * CH)
            base_rep = np.repeat(kpos_base[:, k] * CH, Cbk[:, k] * CH)
            posrel = np.arange(int(Ck_tot[k]) * CH) - base_rep
            arr[posrel >= maxc_rep] = -1
            m = ks == k
            kslot = kpos_base[bs[m], k] * CH + j[m]
            arr[kslot] = (cs[m] - k * src_blk).astype(np.int16)
            # wrap: idx i -> [i % 16, i // 16], replicated to 128 partitions
            wrapped = arr.reshape(-1, 16).T          # [16, Ck*8]
            idx_ks.append(np.tile(wrapped, (8, 1)).copy())  # [128, Ck*8]

        per_core.append(
            {
                "rowloc": rowloc.reshape(C_tot, CH).T.copy(),  # [128, C_tot]
                "normv": normv.reshape(C_tot, CH).T.copy(),
                **{f"idx{k}": idx_ks[k] for k in range(nk)},
            }
        )

    return per_core, Cbk, Ck_tot, C_tot, npc, nblk, nk, maxcnt


def _build(n_nodes, src_blk, Cbk, Ck_tot, C_tot, npc, nblk, nk, G, wide=False,
           dblk=128, maxcnt=None):
    """Build the Bass program (shared by all cores).

    wide=True: accumulate two dest blocks per PSUM tile ([128, 256]) and run
    the segment matmuls in float32r, which streams at 1 cyc/row when the
    moving dim is >=256 (vs 4 for fp32) -- 2x PE time per chunk saved.
    """
    nc = bacc.Bacc(None, target_bir_lowering=False)
    dt = mybir.dt
    assert dblk == 128 or not wide
    DW = 256 if wide else dblk  # psum/S width in dest columns

    mm_t = dt.float32r if wide else dt.float32
    x_d = nc.dram_tensor("x", [n_nodes, F], mm_t, kind="ExternalInput")
    w_d = nc.dram_tensor("w", [F, F], dt.float32, kind="ExternalInput")
    iota_d = nc.dram_tensor("iota", [128, DW], dt.float32, kind="ExternalInput")
    bias_d = nc.dram_tensor("biasrep", [128, F], dt.float32, kind="ExternalInput")
    rowloc_d = nc.dram_tensor("rowloc", [128, C_tot], dt.float32, kind="ExternalInput")
    normv_d = nc.dram_tensor("normv", [128, C_tot], dt.float32, kind="ExternalInput")
    idx_d = [
        nc.dram_tensor(f"idx{k}", [128, int(Ck_tot[k]) * 8], dt.int16,
                       kind="ExternalInput")
        for k in range(nk)
    ]
    y_d = nc.dram_tensor("y", [npc, F], dt.float32, kind="ExternalOutput")

    with tile.TileContext(nc) as tc:
        with (
            tc.tile_pool(name="const", bufs=1) as constp,
            tc.tile_pool(name="slab", bufs=10) as slabp,
            tc.tile_pool(name="idxp", bufs=10) as idxp,
            tc.tile_pool(name="sp", bufs=8) as sp,
            tc.tile_pool(name="pre", bufs=3) as prep,
            tc.tile_pool(name="ob", bufs=3) as obp,
            tc.tile_pool(name="pagg", bufs=4, space="PSUM") as paggp,
            tc.tile_pool(name="pout", bufs=2, space="PSUM") as poutp,
        ):
            w_sb = constp.tile([F, F], dt.float32, tag="w")
            nc.sync.dma_start(w_sb[:], w_d[:])
            iota_sb = constp.tile([128, DW], dt.float32, tag="iota")
            nc.sync.dma_start(iota_sb[:], iota_d[:])
            bias_sb = constp.tile([128, F], dt.float32, tag="bias")
            nc.sync.dma_start(bias_sb[:], bias_d[:])
            rowloc_sb = constp.tile([128, C_tot], dt.float32, tag="rowloc")
            nc.sync.dma_start(rowloc_sb[:], rowloc_d[:])
            normv_sb = constp.tile([128, C_tot], dt.float32, tag="normv")
            nc.sync.dma_start(normv_sb[:], normv_d[:])

            x_src = [x_d[k * src_blk: min((k + 1) * src_blk, n_nodes), :]
                     for k in range(nk)]

            # Pre-zero all slab slots: trailing -1 gather idxs skip the
            # transfer, so skipped slots read stale slot data; generation-0
            # slots would otherwise be uninitialized (NaN*0=NaN in PSUM).
            maxcbk = int(Cbk.max())
            for _ in range(10):
                t0 = slabp.tile([128, maxcbk, F], mm_t, tag="slab")
                z_ap = t0[:] if mm_t == dt.float32 else t0[:].bitcast(dt.float32)
                nc.vector.memset(z_ap, 0.0)

            BPG = 2 if wide else 1   # dest blocks per psum group
            S_BUFS = 8
            if wide:
                # Pre-zero every slot of each half-tag S pool. Chunks of an
                # even/odd block only ever write their own half of an
                # "s0"/"s1" tile, so the other half stays zero across slot
                # generations and the [128, DW] matmul read is safe.
                for h in range(BPG):
                    for _ in range(S_BUFS):
                        t0 = sp.tile([128, DW], mm_t, tag=f"s{h}")
                        nc.vector.memset(t0[:].bitcast(dt.float32), 0.0)

            pos = [0] * nk   # per-k consumed chunk count
            ci = 0           # global chunk index
            for g in range((nblk + BPG - 1) // BPG):
                blocks = [b for b in range(g * BPG, min((g + 1) * BPG, nblk))]
                pa = paggp.tile([128, DW], dt.float32, tag="pagg")
                nchunks = int(sum(Cbk[b].sum() for b in blocks))
                done = 0
                for b in blocks:
                    h = b - g * BPG
                    hs = h * (DW // BPG)
                    for k in range(nk):
                        cbk = int(Cbk[b, k])
                        if cbk == 0:
                            continue
                        # one gather per (b,k) group: trailing -1 idxs (the
                        # chunk-rounding pad beyond the max-core edge count)
                        # transfer nothing; num_idxs_reg = that max count,
                        # identical on every core by construction.
                        slab = slabp.tile([128, cbk, F], mm_t, tag="slab")
                        it = idxp.tile([128, cbk * 8], dt.int16, tag="idx")
                        base = pos[k]
                        nc.sync.dma_start(
                            it[:], idx_d[k][:, base * 8: (base + cbk) * 8]
                        )
                        nc.gpsimd.dma_gather(
                            slab[:],
                            x_src[k],
                            it[:],
                            cbk * CH,
                            int(maxcnt[b, k]),
                            F,
                            # >64 descs in one packet wedges the SDMA engine
                            single_packet=False,
                        )
                        pos[k] += cbk
                        for off in range(cbk):
                            s_t = sp.tile([128, DW], mm_t, tag=f"s{h}")
                            nc.vector.tensor_scalar(
                                s_t[:, hs: hs + dblk],
                                iota_sb[:, :dblk],
                                rowloc_sb[:, ci: ci + 1],
                                normv_sb[:, ci: ci + 1],
                                mybir.AluOpType.is_equal,
                                mybir.AluOpType.mult,
                            )
                            nc.tensor.matmul(
                                pa[:],
                                slab[:, off, :],
                                s_t[:],
                                start=(done == 0),
                                stop=(done == nchunks - 1),
                            )
                            ci += 1
                            done += 1
                # aggT [f, d-group] -> SBUF on scalar engine
                pre = prep.tile([128, DW], dt.float32, tag="pre")
                nc.scalar.activation(
                    pre[:], pa[:], mybir.ActivationFunctionType.Copy
                )
                for b in blocks:
                    h = b - g * BPG
                    hs = h * (DW // BPG)
                    bw = DW // BPG
                    po = poutp.tile([128, F], dt.float32, tag="pout")
                    nc.tensor.matmul(
                        po[:bw, :], pre[:, hs: hs + bw], w_sb[:],
                        start=True, stop=True
                    )
                    ob = obp.tile([128, F], dt.float32, tag="ob")
                    nc.vector.tensor_add(ob[:bw, :], po[:bw, :], bias_sb[:bw, :])
                    rows = min(bw, npc - b * bw)
                    nc.sync.dma_start(
                        y_d[b * bw: b * bw + rows, :], ob[:rows, :]
                    )

    nc.compile()
    return nc


def kernel(x, edge_index, weight, bias, _n_nodes=100000, _src_blk=32768, _g=16,
           _wide=False, _dblk=96, _return_nc=False):
    x = np.ascontiguousarray(np.asarray(x, dtype=np.float32))
    edge_index = np.asarray(edge_index)
    weight = np.ascontiguousarray(np.asarray(weight, dtype=np.float32))
    bias = np.asarray(bias, dtype=np.float32)
    n_nodes = x.shape[0]
    assert n_nodes == _n_nodes and n_nodes % N_CORES == 0

    per_core, Cbk, Ck_tot, C_tot, npc, nblk, nk, maxcnt = _prep(
        x, edge_index, n_nodes, _src_blk, dblk=_dblk
    )
    nc = _build(n_nodes, _src_blk, Cbk, Ck_tot, C_tot, npc, nblk, nk, _g,
                wide=_wide, dblk=_dblk, maxcnt=maxcnt)

    iota = np.tile(np.arange(256 if _wide else _dblk, dtype=np.float32), (128, 1))
    biasrep = np.tile(bias[None, :], (128, 1)).astype(np.float32)
    in_maps = [
        {
            "x": x,
            "w": weight,
            "iota": iota,
            "biasrep": biasrep,
            **per_core[ci],
        }
        for ci in range(N_CORES)
    ]
    res = run_bass_kernel_spmd(nc, in_maps, core_ids=list(range(N_CORES)))
    out = np.concatenate([res.results[ci]["y"] for ci in range(N_CORES)], axis=0)
    if _return_nc:
        return out, nc, in_maps
    return out
